# revision 8
# baseline (speedup 1.0000x reference)
"""Trainium2 Bass kernel for nn_CustomDynaRNN (2-layer GRU + transition/decoder MLPs,
ragged sequence, B=256, T=512).

Strategy:
  - Data-parallel over batch: 8 cores x 32 rows.
  - Feature-major layout on chip: every activation is [features<=128, batch=32];
    weights are stationary lhsT tiles, activations stream as rhs (N=32).
  - Time is processed in chunks of TC steps; inputs/outputs are staged through
    SBUF chunk buffers ([feat, TC*32]) with one big DMA per tensor per chunk.
  - All host-side layout transforms (transposes to feature-major and back) are
    done in numpy, outside the NEFF.
  - softplus(y) (not in the same ACT table as sigmoid/tanh) is computed as
    relu(y) + P(sigmoid(-|y|)) where P(u) ~= -ln(1-u) on (0, 1/2]
    (exact identity: softplus(y) = relu(y) - ln(1 - sigmoid(-|y|))).
"""

import os
import numpy as np

H = 128
L = 2
B = 256
T = 512
EMB = 64
OB = 64
AC = 10
NCORE = 8
BL = B // NCORE  # 32 rows per core

TC = int(os.environ.get("DYNA_TC", "64"))  # timesteps per chunk
WBLOB_COLS = 4 * 384 + 384 + 256 + 128 + 3 * 256 + 2 * 128 + 16 + 4 * 32


# ---------------------------------------------------------------------------
# softplus polynomial: P(u) ~= -ln(1-u) on (0, 0.5], zero constant term.
# Realized as: r = u*A + B; then m times r = (r + a_i)*u.
# Resulting coeffs: u^{m+1}: A, u^m: B + a_1, u^{m-1}: a_2, ..., u^1: a_m.
# ---------------------------------------------------------------------------
_SP_DEG = 6


def _fit_softplus_poly(deg=_SP_DEG):
    x = 0.25 + 0.25 * np.cos(np.linspace(0, np.pi, 8001))
    x = np.clip(x, 1e-9, 0.5)
    y = -np.log1p(-x)
    A = np.stack([x**k for k in range(1, deg + 1)], axis=1)
    w = 1.0 / np.maximum(y, 2e-3)
    c, *_ = np.linalg.lstsq(A * w[:, None], y * w, rcond=None)
    return np.concatenate([[0.0], c])  # coeffs[k] multiplies u^k


_SP_COEF = _fit_softplus_poly()
# chain constants: TS does u*A + B; STT consts a_1..a_m (m = deg-1)
_SP_A = float(_SP_COEF[_SP_DEG])
_SP_B = float(_SP_COEF[_SP_DEG - 1])
_SP_CHAIN = [0.0] + [float(_SP_COEF[k]) for k in range(_SP_DEG - 2, 0, -1)]


def _softplus_poly_ref(u):
    r = u * _SP_A + _SP_B
    for a in _SP_CHAIN:
        r = (r + a) * u
    return r


# ---------------------------------------------------------------------------
# kernel builder
# ---------------------------------------------------------------------------
def build_nc(t_total=T, tc=TC):
    import concourse.bacc as bacc
    import concourse.mybir as mybir
    import concourse.tile as tile

    f32 = mybir.dt.float32
    AF = mybir.ActivationFunctionType
    OP = mybir.AluOpType

    nchunk = t_total // tc
    assert t_total % tc == 0
    FW = tc * BL  # free width of a chunk buffer

    nc = bacc.Bacc(None, target_bir_lowering=False)

    di = lambda n, s: nc.dram_tensor(n, s, f32, kind="ExternalInput")
    do = lambda n, s: nc.dram_tensor(n, s, f32, kind="ExternalOutput")

    d_ob = di("ob_fm", [EMB, t_total * BL])
    d_ac = di("ac_fm", [AC, t_total * BL])   # shifted: col t holds ac_{t+1}
    d_eps = di("eps_fm", [OB, t_total * BL])
    d_mask = di("mask_fm", [1, t_total * BL])
    d_wblob = di("wblob", [128, WBLOB_COLS])  # weights+biases+init

    d_h0o = do("h0_o", [H, t_total * BL])
    d_h1o = do("h1_o", [H, t_total * BL])
    d_muo = do("mu_o", [OB, t_total * BL])
    d_sdo = do("sd_o", [OB, t_total * BL])
    d_obso = do("obs_o", [OB, t_total * BL])

    with tile.TileContext(nc) as tc_ctx:
        with (
            tc_ctx.tile_pool(name="wpool", bufs=1) as wpool,
            tc_ctx.tile_pool(name="inpool", bufs=2) as inpool,
            tc_ctx.tile_pool(name="outpool", bufs=2) as outpool,
            tc_ctx.tile_pool(name="step", bufs=4) as stpool,
            tc_ctx.tile_pool(name="psum", bufs=2, space="PSUM") as ppool,
        ):
            wblob = wpool.tile([128, WBLOB_COLS], f32, tag="wblob")
            nc.sync.dma_start(out=wblob[:], in_=d_wblob[:])
            _c = [0]

            def wslice(cols, rows=128):
                a = wblob[0:rows, _c[0]:_c[0] + cols]
                _c[0] += cols
                return a

            wi0a = wslice(384)
            wh0 = wslice(384)
            wi1 = wslice(384)
            wh1 = wslice(384)
            wi0b = wslice(384, AC)
            wt1 = wslice(256, OB + AC)
            wt2 = wslice(2 * OB)
            wda = wslice(256)
            wdb = wslice(256)
            wdc = wslice(256, AC)
            wmu = wslice(2 * OB)
            wsd = wslice(2 * OB)
            bias = wslice(16)
            init = wslice(4 * BL)

            b_t1 = [bias[:, 0:1], bias[:, 1:2]]
            b_t2 = bias[0:OB, 2:3]
            b_mu = bias[0:OB, 3:4]
            b_sd = bias[0:OB, 4:5]
            b_rz = [[bias[:, 5:6], bias[:, 6:7]], [bias[:, 9:10], bias[:, 10:11]]]
            b_in = [bias[:, 7:8], bias[:, 11:12]]
            b_hn = [bias[:, 8:9], bias[:, 12:13]]
            b_d1 = [bias[:, 13:14], bias[:, 14:15]]

            whh = [wh0, wh1]

            prev = {}

            for c in range(nchunk):
                t0 = c * tc
                xc = inpool.tile([128, FW], f32, tag="xc")       # [nos(0:64); ob(64:128)]
                s1 = inpool.tile([OB + AC, FW], f32, tag="s1")   # [obs; ac_{t+1}]
                acb = inpool.tile([AC, FW], f32, tag="acb")      # ac_{t+1} at rows 0:10
                epsb = inpool.tile([OB, FW], f32, tag="eps")
                mk1 = inpool.tile([1, FW], f32, tag="mk1")
                mkb = inpool.tile([128, FW], f32, tag="mkb")
                h0c = outpool.tile([H, FW], f32, tag="h0c")
                h1c = outpool.tile([H, FW], f32, tag="h1c")
                muc = outpool.tile([OB, FW], f32, tag="muc")
                sdc = outpool.tile([OB, FW], f32, tag="sdc")

                sl = lambda d: d[:, t0 * BL:(t0 + tc) * BL]
                nc.sync.dma_start(out=xc[OB:128, :], in_=sl(d_ob))
                nc.sync.dma_start(out=s1[OB:OB + AC, :], in_=sl(d_ac))
                nc.sync.dma_start(out=acb[:], in_=sl(d_ac))
                nc.sync.dma_start(out=epsb[:], in_=sl(d_eps))
                nc.sync.dma_start(out=mk1[:], in_=sl(d_mask))
                nc.gpsimd.partition_broadcast(mkb[:], mk1[0:1, :])

                for j in range(tc):
                    t = t0 + j
                    S = slice(j * BL, (j + 1) * BL)
                    Sp = slice((j - 1) * BL, j * BL)

                    if t == 0:
                        h0p = init[:, BL:2 * BL]
                        h1p = init[:, 2 * BL:3 * BL]
                        s1p = init[0:OB + AC, 0:BL]
                        acp = init[0:AC, 3 * BL:4 * BL]
                    elif j == 0:
                        pw = (tc - 1) * BL
                        h0p = prev["h0"][:, pw:pw + BL]
                        h1p = prev["h1"][:, pw:pw + BL]
                        s1p = prev["s1"][0:OB + AC, pw:pw + BL]
                        acp = prev["ac"][0:AC, pw:pw + BL]
                    else:
                        h0p = h0c[:, Sp]
                        h1p = h1c[:, Sp]
                        s1p = s1[0:OB + AC, Sp]
                        acp = acb[0:AC, Sp]

                    # psum tiles (4 tags x bufs=2 = 8 banks)
                    pg0 = ppool.tile([128, 4 * BL], f32, tag="pg0")   # r|z|inn|hn
                    pg1 = ppool.tile([128, 4 * BL], f32, tag="pg1")
                    ptn = ppool.tile([128, 3 * BL], f32, tag="ptn")   # t1(2BL) | nos(rows0:64)
                    pdm = ppool.tile([128, 4 * BL], f32, tag="pdm")   # dec(2BL) | mu | sd

                    pno = ptn[0:OB, 2 * BL:3 * BL]
                    pmu = pdm[0:OB, 2 * BL:3 * BL]
                    psd = pdm[0:OB, 3 * BL:4 * BL]

                    _first = {}

                    def mm(out_ap, lhsT, rhs, *, tile_key, last=False):
                        st = tile_key not in _first
                        _first[tile_key] = True
                        nc.tensor.matmul(out_ap, lhsT, rhs, start=st, stop=last,
                                         skip_group_check=True)

                    # h-dependent gate matmuls first (inputs ready from prev step)
                    for li in range(L):
                        pg = (pg0, pg1)[li]
                        hp = (h0p, h1p)[li]
                        wh = whh[li]
                        k = f"pg{li}"
                        mm(pg[:, 0:BL], wh[:, 0:128], hp, tile_key=k)
                        mm(pg[:, BL:2 * BL], wh[:, 128:256], hp, tile_key=k)
                        mm(pg[:, 3 * BL:4 * BL], wh[:, 256:384], hp, tile_key=k)

                    hnb0 = stpool.tile([128, BL], f32, tag="hnb0")
                    hnb1 = stpool.tile([128, BL], f32, tag="hnb1")
                    nc.scalar.activation(hnb0[:], pg0[:, 3 * BL:4 * BL], AF.Identity,
                                         bias=b_hn[0], scale=1.0)
                    nc.scalar.activation(hnb1[:], pg1[:, 3 * BL:4 * BL], AF.Identity,
                                         bias=b_hn[1], scale=1.0)

                    # transition MLP
                    mm(ptn[:, 0:BL], wt1[:, 0:128], s1p, tile_key="ptn")
                    mm(ptn[:, BL:2 * BL], wt1[:, 128:256], s1p, tile_key="ptn")
                    t1h = stpool.tile([128, 2 * BL], f32, tag="t1h")
                    nc.scalar.activation(t1h[:, 0:BL], ptn[:, 0:BL], AF.Tanh,
                                         bias=b_t1[0], scale=1.0)
                    nc.scalar.activation(t1h[:, BL:2 * BL], ptn[:, BL:2 * BL], AF.Tanh,
                                         bias=b_t1[1], scale=1.0)
                    mm(pno, wt2[:, 0:OB], t1h[:, 0:BL], tile_key="ptn")
                    mm(pno, wt2[:, OB:2 * OB], t1h[:, BL:2 * BL], tile_key="ptn", last=True)
                    nc.scalar.activation(xc[0:OB, S], pno, AF.Identity,
                                         bias=b_t2, scale=1.0)

                    # GRU layers
                    nh_prev = None
                    for li in range(L):
                        pg = (pg0, pg1)[li]
                        hp = (h0p, h1p)[li]
                        hnb = (hnb0, hnb1)[li]
                        hc = (h0c, h1c)[li]
                        if li == 0:
                            xin, wia, wib = xc[:, S], wi0a, wi0b
                        else:
                            xin, wia, wib = nh_prev[:], wi1, None
                        k = f"pg{li}"
                        last = wib is None
                        mm(pg[:, 0:BL], wia[:, 0:128], xin, tile_key=k)
                        mm(pg[:, BL:2 * BL], wia[:, 128:256], xin, tile_key=k)
                        mm(pg[:, 2 * BL:3 * BL], wia[:, 256:384], xin, tile_key=k, last=last)
                        if wib is not None:
                            mm(pg[:, 0:BL], wib[:, 0:128], acp, tile_key=k)
                            mm(pg[:, BL:2 * BL], wib[:, 128:256], acp, tile_key=k)
                            mm(pg[:, 2 * BL:3 * BL], wib[:, 256:384], acp, tile_key=k, last=True)

                        srz = stpool.tile([128, 2 * BL], f32, tag=f"srz{li}")
                        nc.scalar.activation(srz[:, 0:BL], pg[:, 0:BL], AF.Sigmoid,
                                             bias=b_rz[li][0], scale=1.0)
                        nc.scalar.activation(srz[:, BL:2 * BL], pg[:, BL:2 * BL], AF.Sigmoid,
                                             bias=b_rz[li][1], scale=1.0)
                        rhn = stpool.tile([128, BL], f32, tag=f"rhn{li}")
                        nc.vector.tensor_tensor(rhn[:], srz[:, 0:BL], hnb[:], OP.mult)
                        npre = stpool.tile([128, BL], f32, tag=f"npre{li}")
                        nc.vector.scalar_tensor_tensor(
                            npre[:], rhn[:], b_in[li], pg[:, 2 * BL:3 * BL],
                            op0=OP.add, op1=OP.add)
                        ngate = stpool.tile([128, BL], f32, tag=f"n{li}")
                        nc.scalar.activation(ngate[:], npre[:], AF.Tanh)
                        dtile = stpool.tile([128, BL], f32, tag=f"d{li}")
                        nc.vector.tensor_tensor(dtile[:], hp, ngate[:], OP.subtract)
                        zd = stpool.tile([128, BL], f32, tag=f"zd{li}")
                        nc.vector.tensor_tensor(zd[:], srz[:, BL:2 * BL], dtile[:], OP.mult)
                        hq = stpool.tile([128, BL], f32, tag=f"hq{li}")
                        nc.vector.tensor_tensor(hq[:], ngate[:], zd[:], OP.add)
                        nc.gpsimd.tensor_tensor(hc[:, S], hq[:], mkb[:, S], OP.mult)
                        nh_prev = hq

                    # decoder
                    mm(pdm[:, 0:BL], wdb[:, 0:128], xc[:, S], tile_key="pdm")
                    mm(pdm[:, BL:2 * BL], wdb[:, 128:256], xc[:, S], tile_key="pdm")
                    mm(pdm[:, 0:BL], wdc[:, 0:128], acp, tile_key="pdm")
                    mm(pdm[:, BL:2 * BL], wdc[:, 128:256], acp, tile_key="pdm")
                    mm(pdm[:, 0:BL], wda[:, 0:128], h1c[:, S], tile_key="pdm")
                    mm(pdm[:, BL:2 * BL], wda[:, 128:256], h1c[:, S], tile_key="pdm")
                    hd = stpool.tile([128, 2 * BL], f32, tag="hd")
                    nc.scalar.activation(hd[:, 0:BL], pdm[:, 0:BL], AF.Tanh,
                                         bias=b_d1[0], scale=1.0)
                    nc.scalar.activation(hd[:, BL:2 * BL], pdm[:, BL:2 * BL], AF.Tanh,
                                         bias=b_d1[1], scale=1.0)

                    # mu / std / sample
                    mm(pmu, wmu[:, 0:OB], hd[:, 0:BL], tile_key="pdm")
                    mm(pmu, wmu[:, OB:2 * OB], hd[:, BL:2 * BL], tile_key="pdm")
                    mm(psd, wsd[:, 0:OB], hd[:, 0:BL], tile_key="pdm")
                    mm(psd, wsd[:, OB:2 * OB], hd[:, BL:2 * BL], tile_key="pdm", last=True)

                    nc.scalar.activation(muc[:, S], pmu, AF.Identity,
                                         bias=b_mu, scale=1.0)
                    rt = stpool.tile([OB, BL], f32, tag="rt")
                    nc.scalar.activation(rt[:], psd, AF.Relu, bias=b_sd, scale=1.0)
                    at = stpool.tile([OB, BL], f32, tag="at")
                    nc.vector.scalar_tensor_tensor(at[:], rt[:], -2.0, psd,
                                                   op0=OP.mult, op1=OP.add)
                    ut = stpool.tile([OB, BL], f32, tag="ut")
                    nc.scalar.activation(ut[:], at[:], AF.Sigmoid, bias=b_sd, scale=1.0)
                    r_cur = stpool.tile([OB, BL], f32, tag="sp0")
                    nc.vector.tensor_scalar(r_cur[:], ut[:], _SP_A, _SP_B,
                                            op0=OP.mult, op1=OP.add)
                    for ki, a in enumerate(_SP_CHAIN):
                        r_nxt = stpool.tile([OB, BL], f32, tag=f"sp{ki + 1}")
                        nc.vector.scalar_tensor_tensor(r_nxt[:], r_cur[:], a, ut[:],
                                                       op0=OP.add, op1=OP.mult)
                        r_cur = r_nxt
                    nc.vector.scalar_tensor_tensor(sdc[:, S], rt[:], 1e-4, r_cur[:],
                                                   op0=OP.add, op1=OP.add)
                    se = stpool.tile([OB, BL], f32, tag="se")
                    nc.vector.tensor_tensor(se[:], sdc[:, S], epsb[:, S], OP.mult)
                    nc.vector.tensor_tensor(s1[0:OB, S], muc[:, S], se[:], OP.add)

                osl = lambda d: d[:, t0 * BL:(t0 + tc) * BL]
                nc.sync.dma_start(out=osl(d_h0o), in_=h0c[:])
                nc.sync.dma_start(out=osl(d_h1o), in_=h1c[:])
                nc.sync.dma_start(out=osl(d_muo), in_=muc[:])
                nc.sync.dma_start(out=osl(d_sdo), in_=sdc[:])
                nc.sync.dma_start(out=osl(d_obso), in_=s1[0:OB, :])

                prev = {"s1": s1, "ac": acb, "h0": h0c, "h1": h1c}

    nc.finalize()
    return nc


# ---------------------------------------------------------------------------
# host-side packing
# ---------------------------------------------------------------------------
def _fm(x):
    """[b, t, f] -> feature-major [f, t*b] float32"""
    return np.ascontiguousarray(x.transpose(2, 1, 0)).reshape(x.shape[2], -1).astype(np.float32)


def _host_pack(inputs, t_total=T):
    ob = np.asarray(inputs["ob_real_emb"], np.float32)
    ac = np.asarray(inputs["ac"], np.float32)
    eps = np.asarray(inputs["noise"], np.float32)
    prev = np.asarray(inputs["prev_hidden_states"], np.float32)
    seq = np.asarray(inputs["sequence_length"]).astype(np.int64)

    w = {k: np.asarray(inputs[k], np.float32) for k in
         ("W_ih0", "W_hh0", "b_ih0", "b_hh0", "W_ih1", "W_hh1", "b_ih1", "b_hh1",
          "Wt1", "bt1", "Wt2", "bt2", "Wd1", "bd1", "Wmu", "bmu", "Wstd", "bstd")}

    wi0T = np.ascontiguousarray(w["W_ih0"].T)  # [138, 384]
    wi0a = np.concatenate([wi0T[EMB:EMB + OB], wi0T[0:EMB]], 0)  # [nos;ob]
    wi0b = wi0T[EMB + OB:]
    wd1 = w["Wd1"]  # [266, 256]
    wdb = np.concatenate([wd1[H:H + OB], wd1[H + OB:H + OB + EMB]], 0)  # [nos;ob]

    bias = np.zeros((128, 16), np.float32)
    bias[:, 0] = w["bt1"][0:128]
    bias[:, 1] = w["bt1"][128:256]
    bias[0:OB, 2] = w["bt2"]
    bias[0:OB, 3] = w["bmu"]
    bias[0:OB, 4] = w["bstd"]
    for li, (bi, bh) in enumerate((("b_ih0", "b_hh0"), ("b_ih1", "b_hh1"))):
        bias[:, 5 + 4 * li] = w[bi][0:128] + w[bh][0:128]
        bias[:, 6 + 4 * li] = w[bi][128:256] + w[bh][128:256]
        bias[:, 7 + 4 * li] = w[bi][256:384]
        bias[:, 8 + 4 * li] = w[bh][256:384]
    bias[:, 13] = w["bd1"][0:128]
    bias[:, 14] = w["bd1"][128:256]

    def pack2(m):  # [256, 64] -> [128, 128] with k-chunk j at cols 64j
        return np.ascontiguousarray(
            m.reshape(2, 128, OB).transpose(1, 0, 2).reshape(128, 2 * OB))

    def pad128(m):
        out = np.zeros((128, m.shape[1]), np.float32)
        out[0:m.shape[0]] = m
        return out

    blob = np.concatenate([
        wi0a, w["W_hh0"].T, w["W_ih1"].T, w["W_hh1"].T,
        pad128(wi0b), pad128(w["Wt1"]), pack2(w["Wt2"]),
        wd1[0:128], wdb, pad128(wd1[256:266]),
        pack2(w["Wmu"]), pack2(w["Wstd"]), bias,
    ], axis=1).astype(np.float32)

    shared = {"wblob": blob}

    tmask = (np.arange(t_total)[:, None] < seq[None, :]).astype(np.float32)  # [T, B]

    in_maps = []
    for core in range(NCORE):
        b0 = core * BL
        bs = slice(b0, b0 + BL)
        ac_sh = np.zeros((BL, t_total, AC), np.float32)
        ac_sh[:, :t_total - 1] = ac[bs, 1:t_total]
        init = np.zeros((128, 4 * BL), np.float32)
        init[0:OB, 0:BL] = prev[bs, 2 * H:].T
        init[OB:OB + AC, 0:BL] = ac[bs, 0].T
        init[:, BL:2 * BL] = prev[bs, 0:H].T
        init[:, 2 * BL:3 * BL] = prev[bs, H:2 * H].T
        init[0:AC, 3 * BL:4 * BL] = ac[bs, 0].T
        blob_c = np.ascontiguousarray(np.concatenate([shared["wblob"], init], axis=1))
        assert blob_c.shape[1] == WBLOB_COLS, blob_c.shape
        m = {
            "ob_fm": _fm(ob[bs, :t_total]),
            "ac_fm": _fm(ac_sh),
            "eps_fm": _fm(eps[bs, :t_total]),
            "mask_fm": np.ascontiguousarray(tmask[:t_total, bs].reshape(1, -1)),
            "wblob": blob_c,
        }
        m.update({k: v for k, v in shared.items() if k != "wblob"})
        in_maps.append(m)
    return in_maps


def _host_unpack(results, t_total=T):
    mus = np.empty((B, t_total, OB), np.float32)
    stds = np.empty((B, t_total, OB), np.float32)
    rets = np.empty((B, t_total, 2 * H + OB), np.float32)

    def un(x, f):
        return x.reshape(f, t_total, BL).transpose(2, 1, 0)

    for core, r in enumerate(results):
        bs = slice(core * BL, (core + 1) * BL)
        mus[bs] = un(r["mu_o"], OB)
        stds[bs] = un(r["sd_o"], OB)
        rets[bs, :, 0:H] = un(r["h0_o"], H)
        rets[bs, :, H:2 * H] = un(r["h1_o"], H)
        rets[bs, :, 2 * H:] = un(r["obs_o"], OB)
    return mus, stds, rets, rets[:, -1, :].copy()


_NC_CACHE = {}


def kernel(**inputs):
    import concourse.bass_utils as bass_utils

    t_total = int(os.environ.get("DYNA_T", str(T)))
    tc = min(TC, t_total)
    key = (t_total, tc)
    if key not in _NC_CACHE:
        _NC_CACHE[key] = build_nc(t_total, tc)
    nc = _NC_CACHE[key]
    in_maps = _host_pack(inputs, t_total)
    res = bass_utils.run_bass_kernel_spmd(
        nc, in_maps, core_ids=list(range(NCORE)),
        trace=bool(int(os.environ.get("DYNA_TRACE", "0"))),
    )
    kernel.last_results = res
    return _host_unpack(res.results, t_total)


def bench(n_iter=3, **inputs):
    """Time the NEFF execution with device-resident inputs (best of n)."""
    import time
    import jax
    import numpy as np_
    from jax.sharding import Mesh, PartitionSpec
    from jax.experimental.shard_map import shard_map
    import concourse.bass2jax as b2j
    import concourse.mybir as mybir

    t_total = int(os.environ.get("DYNA_T", str(T)))
    tc = min(TC, t_total)
    key = (t_total, tc)
    if key not in _NC_CACHE:
        _NC_CACHE[key] = build_nc(t_total, tc)
    nc = _NC_CACHE[key]
    in_maps = _host_pack(inputs, t_total)

    b2j.install_neuronx_cc_hook()
    partition_name = nc.partition_id_tensor.name if nc.partition_id_tensor else None
    in_names, out_names, out_avals, zero_outs = [], [], [], []
    for alloc in nc.m.functions[0].allocations:
        if not isinstance(alloc, mybir.MemoryLocationSet):
            continue
        name = alloc.memorylocations[0].name
        if alloc.kind == "ExternalInput":
            if name != partition_name:
                in_names.append(name)
        elif alloc.kind == "ExternalOutput":
            shape = tuple(alloc.tensor_shape)
            dtype = mybir.dt.np(alloc.dtype)
            out_names.append(name)
            out_avals.append(jax.core.ShapedArray(shape, dtype))
            zero_outs.append(np_.zeros(shape, dtype))
    n_params = len(in_names)
    n_outs = len(out_avals)
    all_names = in_names + out_names + ([partition_name] if partition_name else [])
    donate = tuple(range(n_params, n_params + n_outs))

    def _body(*args):
        operands = list(args)
        if partition_name is not None:
            operands.append(b2j.partition_id_tensor())
        return tuple(b2j._bass_exec_p.bind(
            *operands, out_avals=tuple(out_avals), in_names=tuple(all_names),
            out_names=tuple(out_names), lowering_input_output_aliases=(),
            sim_require_finite=True, sim_require_nnan=True, nc=nc))

    devices = jax.devices()[:NCORE]
    mesh = Mesh(np_.asarray(devices), ("core",))
    sharded = jax.jit(
        shard_map(_body, mesh=mesh,
                  in_specs=(PartitionSpec("core"),) * (n_params + n_outs),
                  out_specs=(PartitionSpec("core"),) * n_outs, check_rep=False),
        donate_argnums=donate, keep_unused=True)

    concat_in = [np_.concatenate([np_.asarray(in_maps[c][n]) for c in range(NCORE)], 0)
                 for n in in_names]
    in_dev = [jax.device_put(x) for x in concat_in]
    for x in in_dev:
        x.block_until_ready()

    times = []
    out = None
    for _ in range(n_iter):
        zs = [jax.device_put(np_.zeros((NCORE * z.shape[0], *z.shape[1:]), z.dtype))
              for z in zero_outs]
        for z in zs:
            z.block_until_ready()
        t0 = time.perf_counter()
        out = sharded(*in_dev, *zs)
        for o in out:
            o.block_until_ready()
        times.append(time.perf_counter() - t0)
    results = [
        {name: np_.asarray(out[i]).reshape(NCORE, *out_avals[i].shape)[c]
         for i, name in enumerate(out_names)}
        for c in range(NCORE)
    ]
    return min(times), times, _host_unpack(results, t_total)
